# revision 2
# baseline (speedup 1.0000x reference)
"""Trainium2 Bass kernel for nn_DPS_topk (topk_masking) — v4.

Forward output is exactly `hard`: the one-hot expansion of the top-16
indices of (logits + gn) along D, k-axis ordered by ascending index
(see kernel.py v1 docstring for the stop_gradient cancellation proof).

One-hotness means 1023/1024 of output bytes are zeros needing no
compute, so HBM write bandwidth binds from t~9us, not DVE.

Structure per core (256 rows = 2 tiles of 128, out = [256, 16K] f32):

  - zero-fill (both HWDGE queues, 425-450 GB/s combined, no compute
    dependency): tile-0 planes 0..11, tile-1 planes 0..7.
  - index path on DVE per tile: x = gn + logits; top-16 positions via
    max8 / find_index8 / match_replace8 twice; positions sorted
    descending (as f32) by a [128,16] max8/match_replace/max8 pass.
  - ones via gpsimd indirect scatter, 4-float granule (HW: the
    scatter writes `in_` rows contiguously at ONE offset per
    partition; a 1-float granule is NRT_EXEC-fatal; a call costs
    ~1.25us serialized on the Q7) -- each (tile, plane) call places a
    16B-aligned block [0..1..0] (one at idx&3) at elem idx&~3, never
    crossing a plane boundary.  20 calls, each gated on the 1-MiB
    zero chunk covering its plane (WAW through HBM), all hidden
    under the remaining stream.
  - dense planes: tile-0 planes 12..15 + tile-1 planes 8..15 are
    produced on DVE (plane j is ONE op: is_equal(column-iota,
    sorted_idx[15-j]) with a per-partition scalar pointer -- no
    cumsum/rank machinery) and DMA'd last, so the stream never waits
    on the Q7 and the final writes are HWDGE with a short receipt.

DVE order c0 -> t0-dense -> c1 -> t1-dense keeps the dense stream fed
(t0's planes cover the first dense slots while c1 still runs) and
releases offs0 early enough to hide all of scatter0.

Raw Bass (no TileContext): one sync-wait condition per instruction;
explicit vector.drain() between dependent same-engine DVE ops; DVE
posted-write slack handled by gating plane DMAs at production index
+1 (v1 trick); iota runs on gpsimd (pattern steps, channel
multiplier, base, and per-partition constants all HW-verified).
"""

import numpy as np

K = 16
D = 1024
N = 64
BS = 32
NCORES = 8
BS_PER_CORE = BS // NCORES   # 4
ROWS = BS_PER_CORE * N       # 256 rows per core
P = 128                      # SBUF partitions
NTILES = ROWS // P           # 2
CH = 2048                    # zero-chunk cols (f32) -> 1 MiB per chunk
NELEM = ROWS * K * D
JD = [12, 8]                 # first dense plane per tile (zeros cover 0..JD-1)
NDENSE = (K - JD[0]) + (K - JD[1])   # 12 dense planes total

_CACHE = {}

# dense planes in DVE production order
DENSE_ORDER = [(0, j) for j in range(JD[0], K)] + [(1, j) for j in range(JD[1], K)]
# pl_sem increments: tile-0 eqs 1..4, drain 5, tile-1 eqs 6..13, final drain 14.
# DMA for a plane waits for the NEXT increment (one-op posted-write slack);
# the last plane of each tile is gated on that tile's drain.
ND0 = K - JD[0]
PL_WAIT = {(0, j): (j - JD[0]) + 2 for j in range(JD[0], K - 1)}
PL_WAIT[(0, K - 1)] = ND0 + 1
PL_WAIT.update({(1, j): ND0 + 1 + (j - JD[1]) + 2 for j in range(JD[1], K - 1)})
PL_WAIT[(1, K - 1)] = ND0 + 2 + (K - JD[1])


def _build_nc():
    from contextlib import ExitStack

    import concourse.bass as bass
    from concourse import mybir

    f32 = mybir.dt.float32
    i32 = mybir.dt.int32
    u32 = mybir.dt.uint32
    A = mybir.AluOpType

    nc = bass.Bass()
    lg_d = nc.declare_dram_parameter("logits", [N, D], f32, isOutput=False)
    gn_d = nc.declare_dram_parameter("gn", [ROWS, D], f32, isOutput=False)
    out_d = nc.declare_dram_parameter("out", [ROWS, K * D], f32, isOutput=True)

    es = ExitStack()

    def sb(name, shape, dt):
        return es.enter_context(nc.sbuf_tensor(name, shape, dt))

    def sem(name):
        return es.enter_context(nc.semaphore(name))

    zeros = sb("zeros", [P, CH], f32)
    gt0 = sb("gt0", [P, D], f32)
    gt1 = sb("gt1", [P, D], f32)
    lg = sb("lg", [P, D], f32)
    x = sb("x", [P, D], f32)
    x2 = sb("x2", [P, D], f32)
    iotaf = sb("iotaf", [P, D], f32)
    chunk = sb("chunk", [P, NDENSE * D], f32)
    v8 = sb("v8", [P, 8], f32)
    v16 = sb("v16", [P, 8], f32)
    i8 = sb("i8", [P, 8], u32)
    i16 = sb("i16", [P, 8], u32)
    sf = sb("sf", [P, K], f32)
    sfr = sb("sfr", [P, K], f32)
    sf2 = sb("sf2", [P, K], f32)
    idx32 = sb("idx32", [P, K], i32)
    al = sb("al", [P, K], i32)
    md = sb("md", [P, K], i32)
    mdf = sb("mdf", [P, K], f32)
    s4f = sb("s4f", [P, 4], f32)
    cm4 = sb("cm4", [P, 1], i32)
    c3 = sb("c3", [P, 1], i32)
    sb_base = [sb("sbase0", [P, K], i32), sb("sbase1", [P, K], i32)]
    offs = [sb("offs0", [P, K], i32), sb("offs1", [P, K], i32)]
    mini = [sb("mini0", [P, 4 * K], f32), sb("mini1", [P, 4 * K], f32)]
    zs_sem = sem("zs_sem")
    in0_sem = sem("in0_sem")
    in1_sem = sem("in1_sem")
    z0a_sem = sem("z0a_sem")   # tile-0 zero chunks, sync queue (c 0,1,2)
    z0b_sem = sem("z0b_sem")   # tile-0 zero chunks, scalar queue (c 3,4,5)
    z1a_sem = sem("z1a_sem")   # tile-1 zero chunks, sync queue (c 0,1)
    z1b_sem = sem("z1b_sem")   # tile-1 zero chunks, scalar queue (c 2,3)
    gp_sem = sem("gp_sem")
    off_sem = sem("off_sem")
    pl_sem = sem("pl_sem")
    pd_sem = sem("pd_sem")
    sc_sem = sem("sc_sem")

    with nc.Block(no_gpsimd_drain=True) as block:

        def zchunk(tile, c):
            return bass.AP(out_d, tile * P * K * D + c * CH, [[K * D, P], [1, CH]])

        def plane_ap(tile, j):
            return bass.AP(out_d, tile * P * K * D + j * D, [[K * D, P], [1, D]])

        def chunk_col(tile, j):
            i = DENSE_ORDER.index((tile, j))
            return chunk[:, i * D : (i + 1) * D]

        @block.sync
        def _(sync: "bass.BassEngine"):
            sync.dma_start(out=lg[0:N, :], in_=lg_d[:, :]).then_inc(in0_sem, 16)
            sync.dma_start(out=gt0[0:N, :], in_=gn_d[0:N, :]).then_inc(in0_sem, 16)
            sync.dma_start(out=gt1[0:N, :], in_=gn_d[P : P + N, :]).then_inc(
                in1_sem, 16
            )
            sync.wait_ge(zs_sem, 1)
            for c in range(3):
                sync.dma_start(out=zchunk(0, c), in_=zeros[:, :]).then_inc(
                    z0a_sem, 16
                )
            for c in range(2):
                sync.dma_start(out=zchunk(1, c), in_=zeros[:, :]).then_inc(
                    z1a_sem, 16
                )
            for tile, j in ((0, 12), (0, 14), (1, 8), (1, 10), (1, 12), (1, 14)):
                sync.wait_ge(pl_sem, PL_WAIT[(tile, j)])
                sync.dma_start(
                    out=plane_ap(tile, j), in_=chunk_col(tile, j)
                ).then_inc(pd_sem, 16)
            sync.wait_ge(z0a_sem, 48)
            sync.wait_ge(z0b_sem, 48)
            sync.wait_ge(z1a_sem, 32)
            sync.wait_ge(z1b_sem, 32)
            sync.wait_ge(pd_sem, 16 * NDENSE)

        @block.scalar
        def _(scalar: "bass.BassEngine"):
            scalar.dma_start(out=lg[N:P, :], in_=lg_d[:, :]).then_inc(in0_sem, 16)
            scalar.dma_start(out=gt0[N:P, :], in_=gn_d[N:P, :]).then_inc(
                in0_sem, 16
            )
            scalar.dma_start(out=gt1[N:P, :], in_=gn_d[P + N : 2 * P, :]).then_inc(
                in1_sem, 16
            )
            scalar.wait_ge(zs_sem, 1)
            for c in range(3, 6):
                scalar.dma_start(out=zchunk(0, c), in_=zeros[:, :]).then_inc(
                    z0b_sem, 16
                )
            for c in range(2, 4):
                scalar.dma_start(out=zchunk(1, c), in_=zeros[:, :]).then_inc(
                    z1b_sem, 16
                )
            for tile, j in ((0, 13), (0, 15), (1, 9), (1, 11), (1, 13), (1, 15)):
                scalar.wait_ge(pl_sem, PL_WAIT[(tile, j)])
                scalar.dma_start(
                    out=plane_ap(tile, j), in_=chunk_col(tile, j)
                ).then_inc(pd_sem, 16)

        @block.gpsimd
        def _(gpsimd: "bass.BassEngine"):
            # scatter element offsets: elem(p, slot) =
            #   (tile*128+p)*16384 + (15-slot)*1024 + idx
            for i in range(NTILES):
                gpsimd.iota(
                    sb_base[i][:, :],
                    pattern=[[-D, K]],
                    base=i * P * K * D + (K - 1) * D,
                    channel_multiplier=K * D,
                )
            gpsimd.iota(s4f[:, :], pattern=[[1, 4]], base=0, channel_multiplier=0,
                        allow_small_or_imprecise_dtypes=True)
            gpsimd.iota(iotaf[:, :], pattern=[[1, D]], base=0, channel_multiplier=0,
                        allow_small_or_imprecise_dtypes=True)
            gpsimd.iota(cm4[:, :], pattern=[[1, 1]], base=-4, channel_multiplier=0)
            gpsimd.iota(c3[:, :], pattern=[[1, 1]], base=3, channel_multiplier=0)
            gpsimd.drain().then_inc(gp_sem, 1)

            def scall(tile, s):
                gpsimd.indirect_dma_start(
                    out=bass.AP(out_d, 0, [[1, NELEM], [1, 1]]),
                    out_offset=bass.IndirectOffsetOnAxis(
                        ap=offs[tile][:, s : s + 1], axis=0
                    ),
                    in_=mini[tile][:, 4 * s : 4 * s + 4],
                    in_offset=None,
                ).then_inc(sc_sem, 16)

            # tile-0 planes 0..11 ascending (slots 15..4), gated per chunk
            gpsimd.wait_ge(off_sem, 1)
            for j in range(0, JD[0]):
                c = j // 2
                if c < 3:
                    gpsimd.wait_ge(z0a_sem, 16 * (c + 1))
                else:
                    gpsimd.wait_ge(z0b_sem, 16 * (c - 2))
                scall(0, K - 1 - j)
            # tile-1 planes 0..7 ascending (slots 15..8)
            gpsimd.wait_ge(off_sem, 2)
            for j in range(0, JD[1]):
                c = j // 2
                if c < 2:
                    gpsimd.wait_ge(z1a_sem, 16 * (c + 1))
                else:
                    gpsimd.wait_ge(z1b_sem, 16 * (c - 1))
                scall(1, K - 1 - j)
            gpsimd.wait_ge(sc_sem, 16 * (JD[0] + JD[1]))

        @block.vector
        def _(vector: "bass.BassEngine"):
            def dr():
                vector.drain()

            vector.memset(zeros[:], 0.0)
            vector.drain().then_inc(zs_sem, 1)
            vector.wait_ge(gp_sem, 1)

            def chain(tile, gt, in_sem, in_tgt, nmini):
                vector.wait_ge(in_sem, in_tgt)
                vector.tensor_tensor(x[:], gt[:], lg[:], op=A.add)
                dr()
                vector.max(v8[:], x[:])
                dr()
                vector.max_index(i8[:], v8[:], x[:])
                vector.match_replace(x2[:], v8[:], x[:], -1e30)
                dr()
                vector.tensor_copy(sf[:, 0:8], i8[:])
                vector.max(v16[:], x2[:])
                dr()
                vector.max_index(i16[:], v16[:], x2[:])
                dr()
                vector.tensor_copy(sf[:, 8:16], i16[:])
                dr()
                vector.max(sf2[:, 0:8], sf[:])
                dr()
                vector.match_replace(sfr[:], sf2[:, 0:8], sf[:], -1.0)
                dr()
                vector.max(sf2[:, 8:16], sfr[:])
                dr()
                vector.tensor_copy(idx32[:], sf2[:])
                dr()
                vector.tensor_tensor(
                    al[:], idx32[:], cm4[:].to_broadcast([P, K]), op=A.bitwise_and
                )
                vector.tensor_tensor(
                    md[:], idx32[:], c3[:].to_broadcast([P, K]), op=A.bitwise_and
                )
                dr()
                vector.tensor_tensor(
                    offs[tile][:], al[:], sb_base[tile][:], op=A.add
                )
                vector.tensor_copy(mdf[:], md[:])
                dr()
                for s in range(K - nmini, K):
                    vector.tensor_scalar(
                        mini[tile][:, 4 * s : 4 * s + 4],
                        s4f[:],
                        mdf[:, s : s + 1],
                        None,
                        op0=A.is_equal,
                    )
                vector.drain().then_inc(off_sem, 1)

            def dense(tile):
                for j in range(JD[tile], K):
                    vector.tensor_scalar(
                        chunk_col(tile, j),
                        iotaf[:],
                        sf2[:, K - 1 - j : K - j],
                        None,
                        op0=A.is_equal,
                    ).then_inc(pl_sem, 1)

            chain(0, gt0, in0_sem, 64, JD[0])
            dense(0)
            vector.drain().then_inc(pl_sem, 1)
            chain(1, gt1, in1_sem, 32, JD[1])
            dense(1)
            vector.drain().then_inc(pl_sem, 1)

    es.close()
    return nc


def _get_nc():
    if "nc" not in _CACHE:
        _CACHE["nc"] = _build_nc()
    return _CACHE["nc"]


def kernel(logits: np.ndarray, gn: np.ndarray) -> np.ndarray:
    from concourse.bass_utils import run_bass_kernel_spmd

    logits = np.ascontiguousarray(np.asarray(logits, dtype=np.float32))
    gn = np.asarray(gn, dtype=np.float32)
    assert logits.shape == (N, D) and gn.shape == (BS, N, D)

    nc = _get_nc()
    in_maps = []
    for c in range(NCORES):
        shard = np.ascontiguousarray(
            gn[c * BS_PER_CORE : (c + 1) * BS_PER_CORE].reshape(ROWS, D)
        )
        in_maps.append({"logits": logits, "gn": shard})

    res = run_bass_kernel_spmd(nc, in_maps, list(range(NCORES))).results
    out = np.concatenate(
        [r["out"].reshape(BS_PER_CORE, N, K, D) for r in res], axis=0
    )
    return out.astype(np.float32, copy=False)


# revision 3
# speedup vs baseline: 1.0859x; 1.0859x over previous
"""Trainium2 Bass kernel for nn_DPS_topk (topk_masking) — v7.

Forward output is exactly `hard`: the one-hot expansion of the top-16
indices of (logits + gn) along D, k-axis ordered by ascending index
(see kernel.py v1 docstring for the stop_gradient cancellation proof).

One-hotness means 1023/1024 of output bytes are zeros needing no
compute, so HBM write bandwidth binds from t~9us, not DVE.

Structure per core (256 rows = 2 tiles of 128, out = [256, 16K] f32):

  - zero-fill (both HWDGE queues, 425-450 GB/s combined, no compute
    dependency): tile-0 planes 0..11, tile-1 planes 0..7.
  - index path on DVE per tile: x = gn + logits; top-16 positions via
    max8 / find_index8 / match_replace8 twice; positions sorted
    descending (as f32) by a [128,16] max8/match_replace/max8 pass.
  - ones via gpsimd indirect scatter, 4-float granule (HW: the
    scatter writes `in_` rows contiguously at ONE offset per
    partition; a 1-float granule is NRT_EXEC-fatal; a call costs
    ~1.25us serialized on the Q7) -- each (tile, plane) call places a
    16B-aligned block [0..1..0] (one at idx&3) at elem idx&~3, never
    crossing a plane boundary.  20 calls, each gated on the 1-MiB
    zero chunk covering its plane (WAW through HBM), all hidden
    under the remaining stream.
  - dense planes: tile-0 planes 12..15 + tile-1 planes 8..15 are
    produced on DVE (plane j is ONE op: is_equal(column-iota,
    sorted_idx[15-j]) with a per-partition scalar pointer -- no
    cumsum/rank machinery) and DMA'd last, so the stream never waits
    on the Q7 and the final writes are HWDGE with a short receipt.

DVE order c0 -> t0-dense -> c1 -> t1-dense keeps the dense stream fed
(t0's planes cover the first dense slots while c1 still runs) and
releases offs0 early enough to hide all of scatter0.

Raw Bass (no TileContext): one sync-wait condition per instruction;
explicit vector.drain() between dependent same-engine DVE ops; DVE
posted-write slack handled by gating plane DMAs at production index
+1 (v1 trick); iota runs on gpsimd (pattern steps, channel
multiplier, base, and per-partition constants all HW-verified).
"""

import numpy as np

K = 16
D = 1024
N = 64
BS = 32
NCORES = 8
BS_PER_CORE = BS // NCORES   # 4
ROWS = BS_PER_CORE * N       # 256 rows per core
P = 128                      # SBUF partitions
NTILES = ROWS // P           # 2
CH = 2048                    # zero-chunk cols (f32) -> 1 MiB per chunk
NELEM = ROWS * K * D
JD = [8, 6]                  # first dense plane per tile (zeros cover 0..JD-1)
NDENSE = (K - JD[0]) + (K - JD[1])   # 12 dense planes total

_CACHE = {}

# dense planes in DVE production order
DENSE_ORDER = [(0, j) for j in range(JD[0], K)] + [(1, j) for j in range(JD[1], K)]
# pl_sem increments: tile-0 eqs 1..10, drain 11, tile-1 eqs 12..21, final drain 22.
# DMA for a plane waits for the NEXT increment (one-op posted-write slack);
# the last plane of each tile is gated on that tile's drain.
ND0 = K - JD[0]
PL_WAIT = {(0, j): (j - JD[0]) + 2 for j in range(JD[0], K - 1)}
PL_WAIT[(0, K - 1)] = ND0 + 1
PL_WAIT.update({(1, j): ND0 + 1 + (j - JD[1]) + 2 for j in range(JD[1], K - 1)})
PL_WAIT[(1, K - 1)] = ND0 + 2 + (K - JD[1])


def _build_nc():
    from contextlib import ExitStack

    import concourse.bass as bass
    from concourse import mybir

    f32 = mybir.dt.float32
    i32 = mybir.dt.int32
    u32 = mybir.dt.uint32
    A = mybir.AluOpType

    nc = bass.Bass()
    lg_d = nc.declare_dram_parameter("logits", [N, D], f32, isOutput=False)
    gn_d = nc.declare_dram_parameter("gn", [ROWS, D], f32, isOutput=False)
    out_d = nc.declare_dram_parameter("out", [ROWS, K * D], f32, isOutput=True)

    es = ExitStack()

    def sb(name, shape, dt):
        return es.enter_context(nc.sbuf_tensor(name, shape, dt))

    def sem(name):
        return es.enter_context(nc.semaphore(name))

    zeros = sb("zeros", [P, CH], f32)
    gt0 = sb("gt0", [P, D], f32)
    gt1 = sb("gt1", [P, D], f32)
    lg = sb("lg", [P, D], f32)
    x = sb("x", [P, D], f32)
    x2 = sb("x2", [P, D], f32)
    iotaf = sb("iotaf", [P, D], f32)
    chunk = sb("chunk", [P, NDENSE * D], f32)
    v8 = sb("v8", [P, 8], f32)
    v16 = sb("v16", [P, 8], f32)
    i8 = sb("i8", [P, 8], u32)
    i16 = sb("i16", [P, 8], u32)
    sf = sb("sf", [P, K], f32)
    sfr = sb("sfr", [P, K], f32)
    sf2 = sb("sf2", [P, K], f32)
    idx32 = sb("idx32", [P, K], i32)
    al = sb("al", [P, K], i32)
    md = sb("md", [P, K], i32)
    mdf = sb("mdf", [P, K], f32)
    s4f = sb("s4f", [P, 4], f32)
    cm4 = sb("cm4", [P, 1], i32)
    c3 = sb("c3", [P, 1], i32)
    sb_base = [sb("sbase0", [P, K], i32), sb("sbase1", [P, K], i32)]
    offs = [sb("offs0", [P, K], i32), sb("offs1", [P, K], i32)]
    mini = [sb("mini0", [P, 4 * K], f32), sb("mini1", [P, 4 * K], f32)]
    zs_sem = sem("zs_sem")
    in0_sem = sem("in0_sem")
    in1_sem = sem("in1_sem")
    z0a_sem = sem("z0a_sem")   # tile-0 zero chunks, sync queue (c 0,1,2)
    z0b_sem = sem("z0b_sem")   # tile-0 zero chunks, scalar queue (c 3,4,5)
    z1a_sem = sem("z1a_sem")   # tile-1 zero chunks, sync queue (c 0,1)
    z1b_sem = sem("z1b_sem")   # tile-1 zero chunks, scalar queue (c 2,3)
    gp_sem = sem("gp_sem")
    off_sem = sem("off_sem")
    pl_sem = sem("pl_sem")
    pd_sem = sem("pd_sem")
    sc_sem = sem("sc_sem")

    with nc.Block(no_gpsimd_drain=True) as block:

        def zchunk(tile, c):
            return bass.AP(out_d, tile * P * K * D + c * CH, [[K * D, P], [1, CH]])

        def plane_ap(tile, j):
            return bass.AP(out_d, tile * P * K * D + j * D, [[K * D, P], [1, D]])

        def chunk_col(tile, j):
            i = DENSE_ORDER.index((tile, j))
            return chunk[:, i * D : (i + 1) * D]

        @block.sync
        def _(sync: "bass.BassEngine"):
            sync.dma_start(out=lg[0:N, :], in_=lg_d[:, :]).then_inc(in0_sem, 16)
            sync.dma_start(out=gt0[0:N, :], in_=gn_d[0:N, :]).then_inc(in0_sem, 16)
            sync.dma_start(out=gt1[0:N, :], in_=gn_d[P : P + N, :]).then_inc(
                in1_sem, 16
            )
            sync.wait_ge(zs_sem, 1)
            for c in (0, 2):
                sync.dma_start(out=zchunk(0, c), in_=zeros[:, :]).then_inc(
                    z0a_sem, 16
                )
            sync.dma_start(out=zchunk(1, 1), in_=zeros[:, :]).then_inc(z1a_sem, 16)
            for tile, j in ((0, 8), (0, 10), (0, 12), (0, 14)):
                sync.wait_ge(pl_sem, PL_WAIT[(tile, j)])
                sync.dma_start(
                    out=plane_ap(tile, j), in_=chunk_col(tile, j)
                ).then_inc(pd_sem, 16)
            for tile, j in ((1, 6), (1, 8), (1, 10), (1, 12), (1, 14)):
                sync.wait_ge(pl_sem, PL_WAIT[(tile, j)])
                sync.dma_start(
                    out=plane_ap(tile, j), in_=chunk_col(tile, j)
                ).then_inc(pd_sem, 16)
            sync.wait_ge(z0a_sem, 32)
            sync.wait_ge(z0b_sem, 32)
            sync.wait_ge(z1a_sem, 16)
            sync.wait_ge(z1b_sem, 32)
            sync.wait_ge(pd_sem, 16 * NDENSE)

        @block.scalar
        def _(scalar: "bass.BassEngine"):
            scalar.dma_start(out=lg[N:P, :], in_=lg_d[:, :]).then_inc(in0_sem, 16)
            scalar.dma_start(out=gt0[N:P, :], in_=gn_d[N:P, :]).then_inc(
                in0_sem, 16
            )
            scalar.dma_start(out=gt1[N:P, :], in_=gn_d[P + N : 2 * P, :]).then_inc(
                in1_sem, 16
            )
            scalar.wait_ge(zs_sem, 1)
            for c in (1, 3):
                scalar.dma_start(out=zchunk(0, c), in_=zeros[:, :]).then_inc(
                    z0b_sem, 16
                )
            scalar.dma_start(out=zchunk(1, 0), in_=zeros[:, :]).then_inc(
                z1b_sem, 16
            )
            for tile, j in ((0, 9), (0, 11), (0, 13), (0, 15)):
                scalar.wait_ge(pl_sem, PL_WAIT[(tile, j)])
                scalar.dma_start(
                    out=plane_ap(tile, j), in_=chunk_col(tile, j)
                ).then_inc(pd_sem, 16)
            scalar.dma_start(out=zchunk(1, 2), in_=zeros[:, :]).then_inc(
                z1b_sem, 16
            )
            for tile, j in ((1, 7), (1, 9), (1, 11), (1, 13), (1, 15)):
                scalar.wait_ge(pl_sem, PL_WAIT[(tile, j)])
                scalar.dma_start(
                    out=plane_ap(tile, j), in_=chunk_col(tile, j)
                ).then_inc(pd_sem, 16)

        @block.gpsimd
        def _(gpsimd: "bass.BassEngine"):
            # scatter element offsets: elem(p, slot) =
            #   (tile*128+p)*16384 + (15-slot)*1024 + idx
            for i in range(NTILES):
                gpsimd.iota(
                    sb_base[i][:, :],
                    pattern=[[-D, K]],
                    base=i * P * K * D + (K - 1) * D,
                    channel_multiplier=K * D,
                )
            gpsimd.iota(s4f[:, :], pattern=[[1, 4]], base=0, channel_multiplier=0,
                        allow_small_or_imprecise_dtypes=True)
            gpsimd.iota(iotaf[:, :], pattern=[[1, D]], base=0, channel_multiplier=0,
                        allow_small_or_imprecise_dtypes=True)
            gpsimd.iota(cm4[:, :], pattern=[[1, 1]], base=-4, channel_multiplier=0)
            gpsimd.iota(c3[:, :], pattern=[[1, 1]], base=3, channel_multiplier=0)
            gpsimd.drain().then_inc(gp_sem, 1)

            def scall(tile, s):
                gpsimd.indirect_dma_start(
                    out=bass.AP(out_d, 0, [[1, NELEM], [1, 1]]),
                    out_offset=bass.IndirectOffsetOnAxis(
                        ap=offs[tile][:, s : s + 1], axis=0
                    ),
                    in_=mini[tile][:, 4 * s : 4 * s + 4],
                    in_offset=None,
                ).then_inc(sc_sem, 16)

            # tile-0 planes 0..11 ascending (slots 15..4), gated per chunk
            gpsimd.wait_ge(off_sem, 1)
            t0_gate = {0: (z0a_sem, 16), 1: (z0b_sem, 16), 2: (z0a_sem, 32), 3: (z0b_sem, 32)}
            for j in range(0, JD[0]):
                gsem, gval = t0_gate[j // 2]
                gpsimd.wait_ge(gsem, gval)
                scall(0, K - 1 - j)
            # tile-1 planes 0..5 ascending (slots 15..10)
            gpsimd.wait_ge(off_sem, 2)
            t1_gate = {0: (z1b_sem, 16), 1: (z1a_sem, 16), 2: (z1b_sem, 32)}
            for j in range(0, JD[1]):
                gsem, gval = t1_gate[j // 2]
                gpsimd.wait_ge(gsem, gval)
                scall(1, K - 1 - j)
            gpsimd.wait_ge(sc_sem, 16 * (JD[0] + JD[1]))

        @block.vector
        def _(vector: "bass.BassEngine"):
            def dr():
                vector.drain()

            vector.memset(zeros[:], 0.0)
            vector.drain().then_inc(zs_sem, 1)
            vector.wait_ge(gp_sem, 1)

            def chain(tile, gt, in_sem, in_tgt, nmini):
                vector.wait_ge(in_sem, in_tgt)
                vector.tensor_tensor(x[:], gt[:], lg[:], op=A.add)
                dr()
                vector.max(v8[:], x[:])
                dr()
                vector.max_index(i8[:], v8[:], x[:])
                vector.match_replace(x2[:], v8[:], x[:], -1e30)
                dr()
                vector.tensor_copy(sf[:, 0:8], i8[:])
                vector.max(v16[:], x2[:])
                dr()
                vector.max_index(i16[:], v16[:], x2[:])
                dr()
                vector.tensor_copy(sf[:, 8:16], i16[:])
                dr()
                vector.max(sf2[:, 0:8], sf[:])
                dr()
                vector.match_replace(sfr[:], sf2[:, 0:8], sf[:], -1.0)
                dr()
                vector.max(sf2[:, 8:16], sfr[:])
                dr()
                vector.tensor_copy(idx32[:], sf2[:])
                dr()
                vector.tensor_tensor(
                    al[:], idx32[:], cm4[:].to_broadcast([P, K]), op=A.bitwise_and
                )
                vector.tensor_tensor(
                    md[:], idx32[:], c3[:].to_broadcast([P, K]), op=A.bitwise_and
                )
                dr()
                vector.tensor_tensor(
                    offs[tile][:], al[:], sb_base[tile][:], op=A.add
                )
                vector.tensor_copy(mdf[:], md[:])
                dr()
                for s in range(K - nmini, K):
                    vector.tensor_scalar(
                        mini[tile][:, 4 * s : 4 * s + 4],
                        s4f[:],
                        mdf[:, s : s + 1],
                        None,
                        op0=A.is_equal,
                    )
                vector.drain().then_inc(off_sem, 1)

            def dense(tile):
                for j in range(JD[tile], K):
                    vector.tensor_scalar(
                        chunk_col(tile, j),
                        iotaf[:],
                        sf2[:, K - 1 - j : K - j],
                        None,
                        op0=A.is_equal,
                    ).then_inc(pl_sem, 1)

            chain(0, gt0, in0_sem, 64, JD[0])
            dense(0)
            vector.drain().then_inc(pl_sem, 1)
            chain(1, gt1, in1_sem, 32, JD[1])
            dense(1)
            vector.drain().then_inc(pl_sem, 1)

    es.close()
    return nc


def _get_nc():
    if "nc" not in _CACHE:
        _CACHE["nc"] = _build_nc()
    return _CACHE["nc"]


def kernel(logits: np.ndarray, gn: np.ndarray) -> np.ndarray:
    from concourse.bass_utils import run_bass_kernel_spmd

    logits = np.ascontiguousarray(np.asarray(logits, dtype=np.float32))
    gn = np.asarray(gn, dtype=np.float32)
    assert logits.shape == (N, D) and gn.shape == (BS, N, D)

    nc = _get_nc()
    in_maps = []
    for c in range(NCORES):
        shard = np.ascontiguousarray(
            gn[c * BS_PER_CORE : (c + 1) * BS_PER_CORE].reshape(ROWS, D)
        )
        in_maps.append({"logits": logits, "gn": shard})

    res = run_bass_kernel_spmd(nc, in_maps, list(range(NCORES))).results
    out = np.concatenate(
        [r["out"].reshape(BS_PER_CORE, N, K, D) for r in res], axis=0
    )
    return out.astype(np.float32, copy=False)
